# revision 1
# baseline (speedup 1.0000x reference)
"""Trainium2 Bass kernel for a batched LSTM + per-step 2-class sigmoid head.

Model (reference): x = concat(points, times) [B,T,24]; LSTM(HS=128) over T;
out = sigmoid(h_seq @ Wc + bc) [B,T,2].  B=512, T=1024.

Strategy: pure data parallel over batch (64 rows per core, 8 cores).
Per core, gate-major on-chip layout:
  - state h,c: [HS=128 partitions, 64 batch]
  - gates for a window of TAU=8 steps live in PSUM as [128, 4*TAU*64] f32,
    one PSUM bank per gate (order i,f,o,g). Window x@W matmuls pre-fill the
    banks (start=True); per-step U@h matmuls accumulate on top (start=False).
  - one Sigmoid ACT covers i,f,o via a strided AP over 3 banks; Tanh for g;
    DVE does c = f*c + i*g; Tanh(c); h = o*tanh(c) written directly as bf16.
  - per-step classifier matmul h@Wc into a rolling PSUM bank; every 256 steps
    one Sigmoid (+bc bias) pass and one contiguous DMA to the output.
Host-side prep: fold bias into W via an all-ones input row, permute gate
columns to (i,f,o,g), transpose x to [25, T*64], cast matmul operands to bf16.
"""

import os
import numpy as np
import ml_dtypes
from contextlib import ExitStack

HS = 128
INP = 23
NCORES = 8

# variant knobs (env-overridable for A/B testing)
CHUNKS = int(os.environ.get("LSTM_CHUNKS", "2"))
FUSE_G = os.environ.get("LSTM_FUSE_G", "1") == "1"
T1_GPSIMD = os.environ.get("LSTM_T1_GPSIMD", "1") == "1"
H_GPSIMD = os.environ.get("LSTM_H_GPSIMD", "1") == "1"
TAU = int(os.environ.get("LSTM_TAU", "4"))

_BUILD_CACHE = {}


def build_lstm(T=1024, BL=64, chunks=1, tau=8, fuse_g=False, t1_gpsimd=False,
               h_gpsimd=False):
    """Build the Bass module for one core (SPMD: all cores identical).

    fuse_g: host pre-scales the g columns of W/U by 2 so one Sigmoid ACT
        covers all 4 gates (sigmoid(2a) = (tanh(a)+1)/2); the c-update uses a
        fused scalar_tensor_tensor to apply g = 2*s - 1.
    t1_gpsimd: compute f*c on GPSIMD to offload the DVE.
    """
    import concourse.bacc as bacc
    import concourse.tile as tile
    from concourse import mybir

    f32 = mybir.dt.float32
    bf16 = mybir.dt.bfloat16
    Sig = mybir.ActivationFunctionType.Sigmoid
    Tanh = mybir.ActivationFunctionType.Tanh

    assert T % tau == 0 and BL % chunks == 0
    CW = BL // chunks
    TB = tau * BL            # free-dim size of one gate's window region
    NW = T // tau
    CLS_STEPS = min(256, T)  # steps per classifier PSUM bank (2 cols per step)
    assert CLS_STEPS % tau == 0 and T % CLS_STEPS == 0

    nc = bacc.Bacc("TRN2", target_bir_lowering=False, debug=False)

    xt_d = nc.dram_tensor("xt", [INP + 2, T * BL], bf16, kind="ExternalInput")
    u_d = nc.dram_tensor("u", [HS, 4 * HS], bf16, kind="ExternalInput")
    w_d = nc.dram_tensor("w", [INP + 2, 4 * HS], bf16, kind="ExternalInput")
    wc_d = nc.dram_tensor("wc", [HS, 2], bf16, kind="ExternalInput")
    bc_d = nc.dram_tensor("bc", [BL, 2], f32, kind="ExternalInput")
    out_d = nc.dram_tensor("out", [BL, T * 2], f32, kind="ExternalOutput")

    with ExitStack() as ctx:
        tc = ctx.enter_context(tile.TileContext(nc))
        consts = ctx.enter_context(tc.tile_pool(name="consts", bufs=1))
        state = ctx.enter_context(tc.tile_pool(name="state", bufs=3))
        gwork = ctx.enter_context(tc.tile_pool(name="gwork", bufs=3))
        outp = ctx.enter_context(tc.tile_pool(name="outp", bufs=2))
        gates_bufs = 2 if tau <= 4 else 1
        gates_ps = ctx.enter_context(
            tc.tile_pool(name="gates_ps", bufs=gates_bufs, space="PSUM"))
        cls_ps = ctx.enter_context(tc.tile_pool(name="cls_ps", bufs=2, space="PSUM"))

        # ---- load constants / inputs into SBUF ----
        xt_sb = consts.tile([INP + 2, T * BL], bf16)
        n_dma = 4
        sl = T * BL // n_dma
        for i in range(n_dma):
            nc.sync.dma_start(
                out=xt_sb[:, i * sl:(i + 1) * sl], in_=xt_d.ap()[:, i * sl:(i + 1) * sl]
            )
        u_sb = consts.tile([HS, 4 * HS], bf16)
        nc.sync.dma_start(out=u_sb, in_=u_d.ap())
        w_sb = consts.tile([INP + 2, 4 * HS], bf16)
        nc.sync.dma_start(out=w_sb, in_=w_d.ap())
        wc_sb = consts.tile([HS, 2], bf16)
        nc.sync.dma_start(out=wc_sb, in_=wc_d.ap())
        bc_sb = consts.tile([BL, 2], f32)
        nc.sync.dma_start(out=bc_sb, in_=bc_d.ap())

        # ---- initial state ----
        c_prev = []
        h_prev = []
        for ch in range(chunks):
            c0 = state.tile([HS, CW], f32, tag=f"c{ch}")
            nc.vector.memset(c0, 0.0)
            h0 = state.tile([HS, CW], bf16, tag=f"h{ch}")
            nc.vector.memset(h0, 0.0)
            c_prev.append(c0)
            h_prev.append(h0)

        from concourse.tile_rust import add_dep_helper

        def phase_a(w, gp):
            # pre-fill window w's gate PSUM banks with x@W (+bias via ones row).
            # start=True clears has_written for the whole bank, so only the
            # first gate region per 2KB bank uses it; later regions in the
            # same bank use start=False (bits clear -> overwrite) and must be
            # ordered after the bank-clearing matmul.
            x_sl = xt_sb[:, w * TB:(w + 1) * TB]
            bank_first = None
            for gc in range(4):
                is_first = (gc * TB * 4) % 2048 == 0
                mm = nc.tensor.matmul(
                    out=gp[:, gc * TB:(gc + 1) * TB],
                    lhsT=w_sb[:, gc * HS:(gc + 1) * HS],
                    rhs=x_sl,
                    start=is_first,
                    stop=False,
                    skip_group_check=True,
                )
                if is_first:
                    bank_first = mm
                else:
                    add_dep_helper(mm.ins, bank_first.ins, sync=False,
                                   reason="bank-clear order")

        mul_engine = nc.gpsimd if t1_gpsimd else nc.vector

        def step_activations(gp_r, s, bsl, ch, c_prev_t):
            if fuse_g:
                # one Sigmoid over all 4 gates; g columns pre-scaled by 2 so
                # slot 3 holds s with tanh(a_g) = 2*s - 1
                sg = gwork.tile([HS, 4, CW], f32, tag=f"sg{ch}")
                nc.scalar.activation(out=sg, in_=gp_r[:, 0:4, s, bsl], func=Sig)
                t1 = gwork.tile([HS, CW], f32, tag=f"t1{ch}")
                mul_engine.tensor_mul(t1, sg[:, 1, :], c_prev_t)
                t2 = gwork.tile([HS, CW], f32, tag=f"t2{ch}")
                nc.vector.tensor_mul(t2, sg[:, 0, :], sg[:, 3, :])   # i * s
                # t3 = 2*(i*s) - i  ( = i * (2s-1) = i * tanh(a_g) )
                t3 = gwork.tile([HS, CW], f32, tag=f"t3{ch}")
                nc.vector.scalar_tensor_tensor(
                    t3, t2, 2.0, sg[:, 0, :],
                    mybir.AluOpType.mult, mybir.AluOpType.subtract,
                )
                c_new = state.tile([HS, CW], f32, tag=f"c{ch}")
                nc.vector.tensor_add(c_new, t1, t3)
            else:
                # sigmoid over i,f,o (strided 3-bank AP), tanh for g
                sg = gwork.tile([HS, 3, CW], f32, tag=f"sg{ch}")
                nc.scalar.activation(out=sg, in_=gp_r[:, 0:3, s, bsl], func=Sig)
                gt = gwork.tile([HS, CW], f32, tag=f"g{ch}")
                nc.scalar.activation(out=gt, in_=gp_r[:, 3, s, bsl], func=Tanh)
                t1 = gwork.tile([HS, CW], f32, tag=f"t1{ch}")
                mul_engine.tensor_mul(t1, sg[:, 1, :], c_prev_t)
                t2 = gwork.tile([HS, CW], f32, tag=f"t2{ch}")
                nc.vector.tensor_mul(t2, sg[:, 0, :], gt)
                c_new = state.tile([HS, CW], f32, tag=f"c{ch}")
                nc.vector.tensor_add(c_new, t1, t2)
            m = gwork.tile([HS, CW], f32, tag=f"m{ch}")
            nc.scalar.activation(out=m, in_=c_new, func=Tanh)
            # h = o * tanh(c), produced directly as bf16 for the matmuls
            h_new = state.tile([HS, CW], bf16, tag=f"h{ch}")
            (nc.gpsimd if h_gpsimd else nc.vector).tensor_mul(h_new, sg[:, 2, :], m)
            return c_new, h_new

        cp = [None] * chunks           # per-chunk classifier PSUM tile
        h_cls = [None] * chunks        # h tile of step t-1 awaiting its cls MM

        def emit_cls(t, ch):
            # classifier matmul for step t (deferred one step so it doesn't
            # sit on the critical chain ahead of the next step's U matmuls)
            r = t % CLS_STEPS
            if r == 0:
                cp[ch] = cls_ps.tile([CW, 2 * CLS_STEPS], f32, tag=f"cp{ch}",
                                     name=f"cp{ch}")
            nc.tensor.matmul(
                out=cp[ch][:, 2 * r:2 * r + 2],
                lhsT=h_cls[ch],
                rhs=wc_sb,
                start=(r == 0),
                stop=(r == CLS_STEPS - 1),
                skip_group_check=True,
            )
            if r == CLS_STEPS - 1:
                # end of a classifier block: sigmoid(+bc) and DMA out
                blk = t // CLS_STEPS
                ob = outp.tile([CW, 2 * CLS_STEPS], f32, tag=f"ob{ch}")
                cp_r = cp[ch].rearrange("p (s c) -> p s c", c=2)
                ob_r = ob.rearrange("p (s c) -> p s c", c=2)
                for cls in range(2):
                    nc.scalar.activation(
                        out=ob_r[:, :, cls],
                        in_=cp_r[:, :, cls],
                        func=Sig,
                        bias=bc_sb[0:CW, cls:cls + 1],
                    )
                nc.sync.dma_start(
                    out=out_d.ap()[ch * CW:(ch + 1) * CW,
                                   blk * 2 * CLS_STEPS:(blk + 1) * 2 * CLS_STEPS],
                    in_=ob,
                )

        gp_cur = gates_ps.tile([HS, 4 * TB], f32, tag="gates")
        phase_a(0, gp_cur)
        for w in range(NW):
            gp_r = gp_cur.rearrange("p (g s b) -> p g s b", g=4, s=tau)
            gp_next = None
            if w + 1 < NW:
                gp_next = gates_ps.tile([HS, 4 * TB], f32, tag="gates")
            for s in range(tau):
                t = w * tau + s
                for ch in range(chunks):
                    bsl = slice(ch * CW, (ch + 1) * CW)
                    # gates += U.T @ h   (4 gate chunks)
                    for gc in range(4):
                        nc.tensor.matmul(
                            out=gp_cur[:, gc * TB + s * BL + ch * CW:
                                       gc * TB + s * BL + (ch + 1) * CW],
                            lhsT=u_sb[:, gc * HS:(gc + 1) * HS],
                            rhs=h_prev[ch],
                            start=False,
                            stop=(s == tau - 1 and ch == chunks - 1),
                            skip_group_check=True,
                        )
                    # previous step's classifier matmul fills the PE gap here
                    if h_cls[ch] is not None:
                        emit_cls(t - 1, ch)
                    c_new, h_new = step_activations(gp_r, s, bsl, ch, c_prev[ch])
                    c_prev[ch] = c_new
                    h_prev[ch] = h_new
                    h_cls[ch] = h_new
                # double-buffered windows: emit next window's x@W mid-window
                if s == 1 and gp_next is not None and gates_bufs > 1:
                    phase_a(w + 1, gp_next)
            if gp_next is not None and gates_bufs == 1:
                phase_a(w + 1, gp_next)
            if gp_next is not None:
                gp_cur = gp_next
        for ch in range(chunks):
            emit_cls(T - 1, ch)
    nc.compile()
    return nc


def _prep_inputs(points, times, W, U, bias, Wc, bc, T, BL, ncores, fuse_g=False):
    """Host-side prep: permute gates to (i,f,o,g), fold bias via ones row,
    transpose x to [25, T*BL] per core, cast matmul operands to bf16."""
    bf = ml_dtypes.bfloat16
    perm = np.concatenate([np.r_[0:HS], np.r_[HS:2 * HS], np.r_[3 * HS:4 * HS],
                           np.r_[2 * HS:3 * HS]])
    Wp = np.concatenate([W, bias[None, :]], axis=0)[:, perm]      # [25, 512]
    Up = U[:, perm]                                                # [128, 512]
    if fuse_g:
        Wp = Wp.copy()
        Up = Up.copy()
        Wp[:, 3 * HS:] *= 2.0    # g columns now produce 2*a_g
        Up[:, 3 * HS:] *= 2.0
    x = np.concatenate([points, times[..., None]], axis=-1)        # [B, T, 24]

    u_bf = np.ascontiguousarray(Up).astype(bf)
    w_bf = np.ascontiguousarray(Wp).astype(bf)                     # [25, 512]
    wc_bf = np.ascontiguousarray(Wc).astype(bf)
    bc_f = np.ascontiguousarray(np.broadcast_to(bc[None, :], (BL, 2))).astype(np.float32)

    in_maps = []
    for k in range(ncores):
        xs = x[k * BL:(k + 1) * BL, :T]                            # [BL, T, 24]
        xt = np.empty((INP + 2, T * BL), dtype=bf)
        xt[:INP + 1] = xs.transpose(2, 1, 0).reshape(INP + 1, T * BL).astype(bf)
        xt[INP + 1] = np.ones((), dtype=bf)                        # bias ones row
        in_maps.append({"xt": xt, "u": u_bf, "w": w_bf, "wc": wc_bf, "bc": bc_f})
    return in_maps


def kernel(points, times, W, U, bias, Wc, bc, _run_kwargs=None):
    from concourse.bass_utils import run_bass_kernel_spmd

    B, T = times.shape
    BL = B // NCORES
    key = (T, BL, CHUNKS, TAU, FUSE_G, T1_GPSIMD, H_GPSIMD)
    if key not in _BUILD_CACHE:
        _BUILD_CACHE[key] = build_lstm(T=T, BL=BL, chunks=CHUNKS, tau=TAU,
                                       fuse_g=FUSE_G, t1_gpsimd=T1_GPSIMD,
                                       h_gpsimd=H_GPSIMD)
    nc = _BUILD_CACHE[key]

    in_maps = _prep_inputs(points, times, W, U, bias, Wc, bc, T, BL, NCORES,
                           fuse_g=FUSE_G)
    kw = _run_kwargs or {}
    res = run_bass_kernel_spmd(nc, in_maps, core_ids=list(range(NCORES)), **kw)
    out = np.concatenate(
        [r["out"].reshape(BL, T, 2) for r in res.results], axis=0
    ).astype(np.float32)
    if _run_kwargs is not None:
        return out, res
    return out

